# revision 1
# baseline (speedup 1.0000x reference)
"""Trainium2 Bass kernel for CausalAttentiveStatisticsPooling (v2).

Per batch element b (data-parallel over 8 cores):
  c_mean   = cumsum(x)/count, c_std = sqrt(cumsum(x^2)/count - c_mean^2)
  h        = tanh(w1^T [x; c_mean; c_std] + b1); scores = w2^T h + b2 per key
  attn     = causal softmax -> e_j/E_i with e = exp(scores)*mask, E = cumsum(e)
  w_mean_i = R_i*cumsum(e*x)_i, w_var_i = R_i*cumsum(e*x^2)_i - w_mean_i^2
  out      = [sum_i<L w_mean_i/L, sum_i<L sqrt(w_var_i)/L]

v2 layout/structure:
  - x path is bf16 end-to-end (xT/xN/x2/w1/triangular consts) - full-rate
    matmuls with light quantization (<0.5% rel; tolerance 2e-2).
  - Phase-1 evictions fused: sqm=ACT.Square(s1*rcnt), var1=DVE.STT(s2*rcnt-sqm),
    cs=ACT.Sqrt(var1+eps).  c_mean chunk tiles never materialized.
  - c_std transposed for the MLP with DMA-transposes (no PE transposes, no
    PSUM transpose bank).
  - final_mean needs no per-query w_mean eviction: suffix trick
    G_j = sum_{i>=j} finalw_i*R_i, final_mean = sum_j (e_j*G_j)*x_j.
  - hpre eviction = DVE.STT(ph + b1 + Pm_slice) (kills ident matmuls).
  - clip(var,eps): kept as a cheap bf16 4x-mode max op where variance can
    genuinely reach 0 (chunk 0 for c_var; all chunks for w_var); dropped for
    c_var chunks >=1 (variance of >=129 iid samples is bounded away from 0).
"""

import numpy as np
import ml_dtypes

B, C, T, A = 8, 512, 2048, 128
NCH = T // 128  # 16 T-chunks
NEG = -30000.0
EPSC = 1e-12

BF = ml_dtypes.bfloat16

# f32 blob columns
CF_RCNT = 0        # (128,16)
CF_MASKEXP = 16    # (128,16)
CF_B1 = 32         # (128,1)
CF_EPS = 33        # (128,1)
CF_SUTRI16 = 34    # (16,16) rows 0:16
CF_LTRI16 = 50     # (16,16) rows 0:16
CF_ONESC = 66      # (128,1)
NF = 67
# f32r blob columns
CR_TRIL = 0        # (128,128)
CR_FINALW = 128    # (128,16)
CR_ONESC = 144     # (128,1)
NR = 145
# bf16 blob columns
CB_TRIU = 0        # (128,128)
CB_ONESCOLS = 128  # (128,256)
CB_SUTRI48 = 384   # (48,16) rows 0:48
CB_W2 = 400        # (128,1)
CB_ONESC = 401     # (128,1)
CB_SEL8 = 402      # (8,16) rows 0:8
CB_UIND = 418      # (128,33*4) bf16, m=0..3
CB_IND4 = 550      # (4,4) rows 0:4
NB = 554

_CACHE = {}


def _build():
    import concourse.bass as bass
    import concourse.mybir as mybir
    import concourse.tile as tile
    from concourse.tile import add_dep_helper
    from concourse import bacc

    f32 = mybir.dt.float32
    f32r = mybir.dt.float32r
    bf16 = mybir.dt.bfloat16
    AF = mybir.ActivationFunctionType
    OP = mybir.AluOpType

    nc = bacc.Bacc("TRN2", target_bir_lowering=False, debug=False,
                   num_devices=8)

    def din(name, shape, dt):
        return nc.dram_tensor(name, shape, dt, kind="ExternalInput").ap()

    d_xT = din("xT", (T, C), bf16)
    d_xN = din("xN", (C, T), bf16)
    d_w1 = din("w1b", (128, 12 * A), bf16)
    d_cf = din("cstf", (128, NF), f32)
    d_cr = din("cstr", (128, NR), f32r)
    d_cb = din("cstb", (128, NB), bf16)
    d_rb = din("rcntb", (128, T), bf16)
    d_out = nc.dram_tensor("out", (1, 2 * C), f32, kind="ExternalOutput").ap()

    from contextlib import ExitStack
    with tile.TileContext(nc) as tc:
        with ExitStack() as stack:
            def pool(name, bufs, space=None):
                kw = {"space": space} if space else {}
                return stack.enter_context(
                    tc.tile_pool(name=name, bufs=bufs, **kw))
            big = pool("big", 1)
            consts = pool("consts", 1)
            colp = pool("colp", 1)
            sqmp = pool("sqmp", 3)
            v1p = pool("v1p", 4)
            csp = pool("csp", 4)
            natp = pool("natp", 8)
            hp = pool("hp", 4)
            hh = pool("hh", 4)
            wtp = pool("wtp", 6)
            zpp = pool("zpp", 3)
            wsp = pool("wsp", 3)
            v2b = pool("v2b", 1)
            ps_s = pool("ps_s", 2, "PSUM")
            ps_s1 = pool("ps_s1", 2, "PSUM")
            ps_tot = pool("ps_tot", 1, "PSUM")
            ps_ca = pool("ps_ca", 1, "PSUM")
            ps_cb = pool("ps_cb", 1, "PSUM")
            # ---------------- DMAs ----------------
            t_cb = consts.tile([128, NB], bf16)
            nc.sync.dma_start(t_cb, d_cb)
            t_cf = consts.tile([128, NF], f32)
            nc.sync.dma_start(t_cf, d_cf)
            t_xT = big.tile([128, NCH, C], bf16)
            x2 = big.tile([128, NCH, C], bf16)
            d_xT_r = d_xT.rearrange("(n p) c -> p n c", p=128)
            t_xN = big.tile([128, 4, T], bf16)
            d_xN_r = d_xN.rearrange("(n p) t -> p n t", p=128)
            t_w1 = consts.tile([128, 12, A], bf16)
            for q in range(8):
                eng = nc.sync if q % 2 == 0 else nc.gpsimd
                eng.dma_start(t_xT[:, 2 * q:2 * (q + 1), :],
                              d_xT_r[:, 2 * q:2 * (q + 1), :])
            for i in range(NCH):
                nc.vector.tensor_mul(x2[:, i, :], t_xT[:, i, :],
                                     t_xT[:, i, :])
            nc.gpsimd.dma_start(t_w1, d_w1.rearrange("p (n a) -> p n a", n=12))
            nc.gpsimd.dma_start(t_xN[:, :, 0:1024], d_xN_r[:, :, 0:1024])
            nc.gpsimd.dma_start(t_xN[:, :, 1024:2048], d_xN_r[:, :, 1024:2048])
            t_cr = consts.tile([128, NR], f32r)
            nc.sync.dma_start(t_cr, d_cr)
            rbp = consts.tile([128, 4, 512], bf16)
            nc.gpsimd.dma_start(rbp,
                                d_rb.rearrange("p (g t) -> p g t", g=4))

            # const views
            t_triub = t_cb[:, CB_TRIU:CB_TRIU + 128]
            t_onescols = t_cb[:, CB_ONESCOLS:CB_ONESCOLS + 256]
            t_w2b = t_cb[:, CB_W2:CB_W2 + 1]
            t_onescb = t_cb[:, CB_ONESC:CB_ONESC + 1]
            t_sel8 = t_cb[0:8, CB_SEL8:CB_SEL8 + 16]
            # all-ones (1,33) row AT PARTITION 32 (matmul needs matching
            # base partitions): row 32 of triu is 1 for cols >= 32
            t_ones33 = t_cb[32:33, CB_TRIU + 32:CB_TRIU + 65]

            def uind(m):
                return t_cb[:, CB_UIND + 33 * m:CB_UIND + 33 * (m + 1)]

            def ind4_ap(m):
                sl = t_cb[0:4, CB_IND4 + m:CB_IND4 + m + 1]
                return bass.AP(tensor=sl.tensor, offset=sl.offset,
                               ap=[[sl.ap[0][0], 4], [0, 128]])
            t_rcnt = t_cf[:, CF_RCNT:CF_RCNT + 16]
            t_maskexp = t_cf[:, CF_MASKEXP:CF_MASKEXP + 16]
            t_b1 = t_cf[:, CF_B1:CF_B1 + 1]
            t_eps = t_cf[:, CF_EPS:CF_EPS + 1]
            t_sutri16 = t_cf[0:16, CF_SUTRI16:CF_SUTRI16 + 16]
            t_ltri16 = t_cf[0:16, CF_LTRI16:CF_LTRI16 + 16]
            t_onescf = t_cf[:, CF_ONESC:CF_ONESC + 1]
            t_tril = t_cr[:, CR_TRIL:CR_TRIL + 128]
            t_finalw = t_cr[:, CR_FINALW:CR_FINALW + 16]
            t_onescr = t_cr[:, CR_ONESC:CR_ONESC + 1]

            def csel_ap(i, k=NCH):
                sl = t_cb[0:k, CB_SUTRI48 + i:CB_SUTRI48 + i + 1]
                return bass.AP(tensor=sl.tensor, offset=sl.offset,
                               ap=[[sl.ap[0][0], k], [0, 128]])

            def bcast16(sb):
                # (16,1) sbuf column -> (16,128) free-broadcast lhsT
                return bass.AP(tensor=sb.tensor, offset=sb.offset,
                               ap=[[sb.ap[0][0], 16], [0, 128]])

            # ---------------- phase-1 totals (two halves) ----------------
            tot1a = colp.tile([8, C], bf16)
            tot2a = colp.tile([8, C], bf16)
            tot1b = colp.tile([16, C], bf16)
            tot2b = colp.tile([16, C], bf16)
            for half in range(2):
                lo, hi = 8 * half, 8 * half + 8
                ps_t1 = ps_tot.tile([16, C], f32, tag="tX")
                ps_t2 = ps_tot.tile([16, C], f32, tag="tY")
                for i in range(lo, hi):
                    oc = t_onescols[:, 16 * i:16 * (i + 1)]
                    nc.tensor.matmul(ps_t1[:], oc, t_xT[:, i, :],
                                     start=(i == lo),
                                     stop=(half == 0 and i == hi - 1))
                for i in range(lo, hi):
                    oc = t_onescols[:, 16 * i:16 * (i + 1)]
                    nc.tensor.matmul(ps_t2[:], oc, x2[:, i, :],
                                     start=(i == lo),
                                     stop=(half == 0 and i == hi - 1))
                if half == 0:
                    nc.vector.tensor_copy(tot1a[:], ps_t1[0:8, :])
                    nc.vector.tensor_copy(tot2a[:], ps_t2[0:8, :])
                else:
                    nc.tensor.matmul(ps_t1[:], t_sel8, tot1a[:],
                                     start=False, stop=True)
                    nc.tensor.matmul(ps_t2[:], t_sel8, tot2a[:],
                                     start=False, stop=True)
                    nc.vector.tensor_copy(tot1b[:], ps_t1[:])
                    nc.vector.tensor_copy(tot2b[:], ps_t2[:])

            # ---------------- phase 1 + MLP ----------------
            nats = [None] * NCH
            last_cs_inst = None
            for i in range(NCH):
                s1 = ps_s.tile([128, C], f32, tag="sA")
                nc.tensor.matmul(s1[:], t_triub, t_xT[:, i, :],
                                 start=True, stop=(i == 0))
                s2 = ps_s1.tile([128, C], f32, tag="sB")
                nc.tensor.matmul(s2[:], t_triub, x2[:, i, :],
                                 start=True, stop=(i == 0))
                if 0 < i <= 8:
                    nc.tensor.matmul(s1[:], csel_ap(i, 8), tot1a[:],
                                     start=False, stop=True)
                    nc.tensor.matmul(s2[:], csel_ap(i, 8), tot2a[:],
                                     start=False, stop=True)
                elif i > 8:
                    nc.tensor.matmul(s1[:], csel_ap(i), tot1b[:],
                                     start=False, stop=True)
                    nc.tensor.matmul(s2[:], csel_ap(i), tot2b[:],
                                     start=False, stop=True)
                sqm = sqmp.tile([128, C], f32, tag="sqm")
                nc.scalar.activation(sqm[:], s1[:], AF.Square,
                                     scale=t_rcnt[:, i:i + 1])
                var1 = v1p.tile([128, C], bf16, tag="v1")
                nc.vector.scalar_tensor_tensor(var1[:], s2[:],
                                               t_rcnt[:, i:i + 1], sqm[:],
                                               op0=OP.mult, op1=OP.subtract)
                if i == 0:
                    # only chunk 0 can see true variance near 0 (count<128);
                    # later chunks have >=129-sample variance, far from 0
                    nc.vector.tensor_scalar_max(var1[:], var1[:], EPSC)
                cs = csp.tile([128, C], bf16, tag="cs")
                cs_inst = nc.scalar.activation(cs[:], var1[:], AF.Sqrt)
                last_cs_inst = cs_inst
                nat = natp.tile([128, 4, 128], bf16, tag="nat")
                nc.sync.dma_start_transpose(nat[:], cs[:])
                nats[i] = nat

            # ---------------- MLP (after p1 chunks; fills the tanh gap) ---
            Pm_sb = big.tile([128, T], f32r)
            zeros512 = consts.tile([128, 512], f32)
            nc.vector.memset(zeros512[:], 0.0)
            hpres = [None] * 4
            for g in range(4):
                pm_ps = ps_ca.tile([A, 512], f32, tag="cA")
                for cb in range(4):
                    nc.tensor.matmul(pm_ps[:], t_w1[:, 4 + cb, :],
                                     t_xN[:, cb, 512 * g:512 * (g + 1)],
                                     start=(cb == 0), stop=(cb == 3))
                sl = Pm_sb[:, 512 * g:512 * (g + 1)]
                init = (0.0 if g == 0
                        else Pm_sb[:, 512 * g - 1:512 * g].bitcast(f32))
                nc.vector.tensor_tensor_scan(sl, pm_ps[:], zeros512[:],
                                             initial=init,
                                             op0=OP.add, op1=OP.add)
                nc.vector.tensor_mul(sl, sl.bitcast(f32), rbp[:, g, :])
                ph = ps_tot.tile([A, 512], f32, tag=("tX" if g % 2 == 0
                                                     else "tY"))
                for cb in range(4):
                    nc.tensor.matmul(ph[:], t_w1[:, cb, :],
                                     t_xN[:, cb, 512 * g:512 * (g + 1)],
                                     start=(cb == 0), stop=False)
                for k in range(4):
                    for cb in range(4):
                        nc.tensor.matmul(
                            ph[:, 128 * k:128 * (k + 1)],
                            t_w1[:, 8 + cb, :],
                            nats[4 * g + k][:, cb, :],
                            start=False, stop=(cb == 3))
                hpre = hp.tile([A, 512], f32, tag="hpre")
                nc.vector.scalar_tensor_tensor(
                    hpre[:], ph[:], t_b1,
                    Pm_sb[:, 512 * g:512 * (g + 1)].bitcast(f32),
                    op0=OP.add, op1=OP.add)
                hpres[g] = hpre

            # E accumulates in its own PSUM tile (start=True clears whole
            # 32-partition strips bank-wide, so scores/etot cannot share it)
            eEp = ps_cb.tile([128, NCH], f32, tag="cB")
            eTf = colp.tile([128, NCH], f32)
            eTb = colp.tile([128, NCH], bf16)
            nc.vector.memset(eTb[:], 0.0)
            eR = colp.tile([128, NCH], f32)
            etot = colp.tile([16, 1], f32)
            R_col = colp.tile([128, NCH], f32)
            wtris = []
            cumMs, cumAs = [], []
            var2s = [None] * NCH
            last_exp_inst = None
            # loop A: tanh/scores/exp + E/R per group (clears the sqrt-table
            # gate early so ws overlaps the heavy loop below)
            for g in range(4):
                h = hh.tile([A, 512], bf16, tag="h_sb")
                tanh_inst = nc.scalar.activation(h[:], hpres[g][:], AF.Tanh)
                add_dep_helper(tanh_inst.ins, last_cs_inst.ins, sync=False,
                               reason="keep exp-table ACT ops after all sqrts")
                ps_sc = ps_ca.tile([128, 4], f32, tag="cA")
                for k in range(4):
                    nc.tensor.matmul(
                        ps_sc[:, k:k + 1],
                        h[:, 128 * k:128 * (k + 1)],
                        t_w2b, start=True, stop=True)
                nc.vector.tensor_add(eTf[:, 4 * g:4 * g + 4],
                                     ps_sc[:],
                                     t_maskexp[:, 4 * g:4 * g + 4])
                last_exp_inst = nc.scalar.activation(
                    eTb[:, 4 * g:4 * g + 4], eTf[:, 4 * g:4 * g + 4], AF.Exp)
                nc.vector.tensor_copy(eR[:, 4 * g:4 * g + 4],
                                      eTb[:, 4 * g:4 * g + 4])
                ps_et = ps_ca.tile([16, 1], f32, tag="cA")
                nc.tensor.matmul(ps_et[:], eTb[:], t_onescb,
                                 start=True, stop=True)
                nc.vector.tensor_copy(etot[:], ps_et[:])
                nc.tensor.matmul(eEp[:, 4 * g:4 * g + 4], t_triub,
                                 eTb[:, 4 * g:4 * g + 4],
                                 start=True, stop=False)
                nc.tensor.matmul(eEp[:, 4 * g:4 * g + 4], bcast16(etot),
                                 t_sutri16[:, 4 * g:4 * g + 4],
                                 start=False, stop=True)
                nc.vector.reciprocal(R_col[:, 4 * g:4 * g + 4],
                                     eEp[:, 4 * g:4 * g + 4])

            # ---------------- G (suffix weights for final mean) -----------
            RLr = colp.tile([128, NCH], f32r)
            nc.vector.tensor_mul(RLr[:], R_col[:], t_finalw.bitcast(f32))
            ps_G = ps_cb.tile([128, NCH], f32, tag="cB")
            nc.tensor.matmul(ps_G[:], t_tril, RLr[:], start=True, stop=False)
            ps_rt = ps_ca.tile([16, 1], f32, tag="cA")
            nc.tensor.matmul(ps_rt[:], RLr[:].bitcast(f32), t_onescf,
                             start=True, stop=True)
            rtot = colp.tile([16, 1], f32)
            nc.vector.tensor_copy(rtot[:], ps_rt[:])
            nc.tensor.matmul(ps_G[:], bcast16(rtot), t_ltri16, start=False,
                             stop=True)
            wG = colp.tile([128, NCH], bf16)
            nc.vector.tensor_mul(wG[:], eTb[:], ps_G[:])

            # loop B: weights, cumulative totals, phase-2 chains, ws/fs
            ps_fm = ps_ca.tile([1, C], f32, tag="cA")
            ps_fs = ps_cb.tile([1, C], f32, tag="cB")
            first_done = False
            for g in range(4):
                ps_cM = ps_tot.tile([33, C], f32, tag="tX")
                ps_cA = ps_tot.tile([33, C], f32, tag="tY")
                for kk in range(4):
                    ii = 4 * g + kk
                    wtri = wtp.tile([128, 128], bf16, tag="wtri")
                    nc.vector.tensor_scalar_mul(
                        wtri[:], t_triub, eR[:, ii:ii + 1])
                    wtris.append(wtri)
                    wecol = wtp.tile([128, 33], bf16, tag="wecol")
                    nc.vector.tensor_scalar_mul(
                        wecol[:], uind(kk), eR[:, ii:ii + 1])
                    nc.tensor.matmul(ps_cM[:], wecol[:], t_xT[:, ii, :],
                                     start=(kk == 0),
                                     stop=(g == 0 and kk == 3))
                    nc.tensor.matmul(ps_cA[:], wecol[:], x2[:, ii, :],
                                     start=(kk == 0),
                                     stop=(g == 0 and kk == 3))
                if g > 0:
                    nc.tensor.matmul(ps_cM[:], t_ones33, cumMs[g - 1][32:33, :],
                                     start=False, stop=True)
                    nc.tensor.matmul(ps_cA[:], t_ones33, cumAs[g - 1][32:33, :],
                                     start=False, stop=True)
                cumM = colp.tile([33, C], bf16, tag=f"cumM{g}")
                nc.vector.tensor_copy(cumM[:], ps_cM[:])
                cumMs.append(cumM)
                cumA = colp.tile([33, C], bf16, tag=f"cumA{g}")
                nc.vector.tensor_copy(cumA[:], ps_cA[:])
                cumAs.append(cumA)
                for kk in range(4):
                    ii = 4 * g + kk
                    mp = ps_s.tile([128, C], f32, tag="sA")
                    nc.tensor.matmul(mp[:], wtris[ii][:], t_xT[:, ii, :],
                                     start=True, stop=(ii == 0))
                    if ii > 0:
                        nc.tensor.matmul(mp[:], ind4_ap(kk), cumM[0:4, :],
                                         start=False, stop=True)
                    ap = ps_s1.tile([128, C], f32, tag="sB")
                    nc.tensor.matmul(ap[:], wtris[ii][:], x2[:, ii, :],
                                     start=True, stop=(ii == 0))
                    if ii > 0:
                        nc.tensor.matmul(ap[:], ind4_ap(kk), cumA[0:4, :],
                                         start=False, stop=True)
                    zp = zpp.tile([128, C], f32, tag="zp")
                    nc.scalar.activation(zp[:], mp[:], AF.Square,
                                         scale=R_col[:, ii:ii + 1])
                    var2 = v2b.tile([128, C], bf16, tag=f"v2_{ii}")
                    nc.vector.scalar_tensor_tensor(var2[:], ap[:],
                                                   R_col[:, ii:ii + 1], zp[:],
                                                   op0=OP.mult,
                                                   op1=OP.subtract)
                    nc.gpsimd.tensor_scalar_max(var2[:], var2[:], EPSC)
                    ws = wsp.tile([128, C], f32r, tag="ws")
                    ws_inst = nc.scalar.activation(ws[:], var2[:], AF.Sqrt)
                    add_dep_helper(ws_inst.ins, last_exp_inst.ins, sync=False,
                                   reason="sqrt-table reload only after exps")
                    nc.tensor.matmul(ps_fm[:], wG[:, ii:ii + 1],
                                     t_xT[:, ii, :],
                                     start=(ii == 0), stop=(ii == NCH - 1))
                    nc.tensor.matmul(ps_fs[:], t_finalw[:, ii:ii + 1], ws[:],
                                     start=(ii == 0), stop=(ii == NCH - 1))

            out_sb = colp.tile([1, 2 * C], f32)
            nc.vector.tensor_copy(out_sb[:, 0:C], ps_fm[:])
            nc.vector.tensor_copy(out_sb[:, C:2 * C], ps_fs[:])
            nc.sync.dma_start(d_out, out_sb[:])

    nc.compile()
    return nc


def _host_inputs(x, lengths, w1, b1, w2, b2):
    x = np.asarray(x, np.float32)
    lengths = np.asarray(lengths)
    w1 = np.asarray(w1, np.float32)
    b1 = np.asarray(b1, np.float32)
    w2 = np.asarray(w2, np.float32)
    b2 = np.asarray(b2, np.float32)

    cstf0 = np.zeros((128, NF), np.float32)
    cstf0[:, CF_B1] = b1
    cstf0[:, CF_EPS] = EPSC
    sutri16 = np.triu(np.ones((16, 16), np.float32), 1)
    cstf0[0:16, CF_SUTRI16:CF_SUTRI16 + 16] = sutri16
    cstf0[0:16, CF_LTRI16:CF_LTRI16 + 16] = np.tril(
        np.ones((16, 16), np.float32), -1)
    cstf0[:, CF_ONESC] = 1.0

    cstr = np.zeros((128, NR), np.float32)
    cstr[:, CR_TRIL:CR_TRIL + 128] = np.tril(np.ones((128, 128), np.float32))
    cstr[:, CR_ONESC] = 1.0

    cstb = np.zeros((128, NB), np.float32)
    cstb[:, CB_TRIU:CB_TRIU + 128] = np.triu(np.ones((128, 128), np.float32))
    onescols = np.zeros((128, 256), np.float32)
    for i in range(NCH):
        onescols[:, 16 * i + i] = 1.0
    cstb[:, CB_ONESCOLS:CB_ONESCOLS + 256] = onescols
    sutri48 = np.zeros((48, 16), np.float32)
    sutri48[0:16] = sutri16
    sutri48[32:48] = sutri16
    cstb[0:48, CB_SUTRI48:CB_SUTRI48 + 16] = sutri48
    cstb[0:8, CB_SEL8:CB_SEL8 + 16] = np.eye(8, 16, dtype=np.float32)
    for m in range(4):
        u = np.zeros((128, 33), np.float32)
        for r in range(4):
            if r > m:
                u[:, r] = 1.0
        u[:, 32] = 1.0
        cstb[:, CB_UIND + 33 * m:CB_UIND + 33 * (m + 1)] = u
    cstb[0:4, CB_IND4:CB_IND4 + 4] = np.eye(4, dtype=np.float32)
    cstb[:, CB_W2] = w2[:, 0]
    cstb[:, CB_ONESC] = 1.0
    cstb = cstb.astype(BF)

    tt = np.arange(T)
    w1b = np.ascontiguousarray(
        w1.reshape(12, 128, A).transpose(1, 0, 2).reshape(128, 12 * A)
    ).astype(BF)

    maps = []
    for b in range(B):
        L = int(lengths[b])
        rcnt = (1.0 / np.minimum(tt + 1, max(L, 1))).astype(np.float32)
        maskexp = (float(b2[0]) +
                   np.where(tt < L, 0.0, NEG)).astype(np.float32)
        finalw = np.where(tt < L, 1.0 / max(L, 1), 0.0).astype(np.float32)
        cstf = cstf0.copy()
        cstf[:, CF_RCNT:CF_RCNT + 16] = rcnt.reshape(NCH, 128).T
        cstf[:, CF_MASKEXP:CF_MASKEXP + 16] = maskexp.reshape(NCH, 128).T
        cstr_b = cstr.copy()
        cstr_b[:, CR_FINALW:CR_FINALW + 16] = finalw.reshape(NCH, 128).T
        maps.append({
            "xT": np.ascontiguousarray(x[b].T).astype(BF),
            "xN": np.ascontiguousarray(x[b]).astype(BF),
            "w1b": w1b,
            "cstf": cstf,
            "cstr": cstr_b,
            "cstb": cstb,
            "rcntb": np.ascontiguousarray(
                np.broadcast_to(rcnt[None, :], (128, T))).astype(BF),
        })
    return maps


def kernel(x, lengths, w1, b1, w2, b2):
    from concourse.bass_utils import run_bass_kernel_spmd

    if "nc" not in _CACHE:
        _CACHE["nc"] = _build()
    nc = _CACHE["nc"]
    maps = _host_inputs(x, lengths, w1, b1, w2, b2)
    res = run_bass_kernel_spmd(nc, maps, list(range(B))).results
    out = np.stack([res[b]["out"][0] for b in range(B)], axis=0)
    return out.astype(np.float32)



# revision 11
# speedup vs baseline: 1.0876x; 1.0876x over previous
"""Trainium2 Bass kernel for CausalAttentiveStatisticsPooling (v3).

Per batch element b (data-parallel over 8 cores):
  c_mean   = cumsum(x)/count, c_std = sqrt(cumsum(x^2)/count - c_mean^2)
  h        = tanh(w1^T [x; c_mean; c_std] + b1); scores = w2^T h + b2 per key
  attn     = causal softmax -> e_j/E_i with e = exp(scores)*mask, E = cumsum(e)
  w_mean_i = R_i*cumsum(e*x)_i, w_var_i = R_i*cumsum(e*x^2)_i - w_mean_i^2
  out      = [sum_i<L w_mean_i/L, sum_i<L sqrt(w_var_i)/L]

v3 structure (cost-model-driven):
  - All "total"/reduction matmuls are tall-skinny (output free size 1):
    per-chunk totals of x/x^2/(e*x)/(e*x^2) land as (C-part, chunk) columns
    via 128-high lhsT blocks; PE transposes (+identity) turn them into
    (16, C) carry rows for the chunk-carry broadcast-add matmuls.
  - final_mean/final_std are tall-skinny accumulations into a (128,8)
    PSUM tile (4 c-blocks x {mean,std}), transposed once at the end.
  - Squares moved off ACT: sqm = (s1*rcnt2)*s1 on Pool STT; var on DVE STT.
  - ACT only does sqrt/tanh/exp + small copies.
  - No explicit act-table loads (walrus re-inserts them at NEFF lowering).
"""

import numpy as np
import ml_dtypes

B, C, T, A = 8, 512, 2048, 128
NCH = T // 128  # 16 T-chunks
NEG = -30000.0
EPSC = 1e-12

BF = ml_dtypes.bfloat16

# f32 blob columns
CF_RCNT = 0         # (128,16)
CF_RCNT2 = 16       # (128,16)
CF_MASKEXP = 32     # (128,16)
CF_B1 = 48          # (128,1)
CF_EPS = 49         # (128,1)
CF_SUTRI16 = 50     # (16,16) rows 0:16
CF_LTRI16 = 66      # (16,16) rows 0:16
CF_ONESC = 82       # (128,1)
CF_IDENT = 83       # (128,128)
NF = 211
# f32r blob columns
CR_TRIL = 0         # (128,128)
CR_FINALW = 128     # (128,16)
CR_ONESC = 144      # (128,1)
NR = 145
# bf16 blob columns
CB_TRIU = 0         # (128,128)
CB_SUTRI16B = 128   # (16,16) rows 0:16
CB_W2 = 144         # (128,1)
CB_ONESC = 145      # (128,1)
CB_IDENTB = 146     # (128,128)
CB_FWB = 274        # (128,16)
NB = 290

_CACHE = {}


def _build():
    import concourse.bass as bass
    import concourse.mybir as mybir
    import concourse.tile as tile
    from concourse import bacc

    f32 = mybir.dt.float32
    f32r = mybir.dt.float32r
    bf16 = mybir.dt.bfloat16
    AF = mybir.ActivationFunctionType
    OP = mybir.AluOpType

    nc = bacc.Bacc("TRN2", target_bir_lowering=False, debug=False,
                   num_devices=8)

    def din(name, shape, dt):
        return nc.dram_tensor(name, shape, dt, kind="ExternalInput").ap()

    d_xT = din("xT", (T, C), bf16)
    d_xN = din("xN", (C, T), bf16)
    d_w1 = din("w1b", (128, 12 * A), bf16)
    d_cf = din("cstf", (128, NF), f32)
    d_cr = din("cstr", (128, NR), f32r)
    d_cb = din("cstb", (128, NB), bf16)
    d_rb = din("rcntb", (128, T), bf16)
    d_out = nc.dram_tensor("out", (1, 2 * C), f32, kind="ExternalOutput").ap()

    from contextlib import ExitStack
    with tile.TileContext(nc) as tc:
        with ExitStack() as stack:
            def pool(name, bufs, space=None):
                kw = {"space": space} if space else {}
                return stack.enter_context(
                    tc.tile_pool(name=name, bufs=bufs, **kw))
            big = pool("big", 1)
            consts = pool("consts", 1)
            colp = pool("colp", 1)
            sqmp = pool("sqmp", 3)
            v1p = pool("v1p", 4)
            csp = pool("csp", 4)
            natp = pool("natp", 8)
            hp = pool("hp", 4)
            hh = pool("hh", 4)
            wtp = pool("wtp", 6)
            zpp = pool("zpp", 3)
            v2b = pool("v2b", 3)
            rowp = pool("rowp", 1)
            ps_s = pool("ps_s", 2, "PSUM")
            ps_s1 = pool("ps_s1", 2, "PSUM")
            ps_tot = pool("ps_tot", 1, "PSUM")
            ps_ca = pool("ps_ca", 1, "PSUM")
            ps_cb = pool("ps_cb", 1, "PSUM")
            # ---------------- DMAs ----------------
            t_cb = consts.tile([128, NB], bf16)
            nc.sync.dma_start(t_cb, d_cb)
            t_cf = consts.tile([128, NF], f32)
            nc.sync.dma_start(t_cf, d_cf)
            t_xT = big.tile([128, NCH, C], bf16)
            x2 = big.tile([128, NCH, C], bf16)
            d_xT_r = d_xT.rearrange("(n p) c -> p n c", p=128)
            t_xN = big.tile([128, 4, T], bf16)
            d_xN_r = d_xN.rearrange("(n p) t -> p n t", p=128)
            t_w1 = consts.tile([128, 12, A], bf16)
            for q in range(8):
                eng = nc.sync if q % 2 == 0 else nc.scalar
                eng.dma_start(t_xT[:, 2 * q:2 * (q + 1), :],
                              d_xT_r[:, 2 * q:2 * (q + 1), :])
            for i in range(NCH):
                nc.vector.tensor_mul(x2[:, i, :], t_xT[:, i, :],
                                     t_xT[:, i, :])
            nc.scalar.dma_start(t_w1, d_w1.rearrange("p (n a) -> p n a", n=12))
            nc.scalar.dma_start(t_xN[:, :, 0:1024], d_xN_r[:, :, 0:1024])
            nc.scalar.dma_start(t_xN[:, :, 1024:2048], d_xN_r[:, :, 1024:2048])
            t_cr = consts.tile([128, NR], f32r)
            nc.sync.dma_start(t_cr, d_cr)
            rbp = consts.tile([128, 4, 512], bf16)
            nc.scalar.dma_start(rbp,
                                d_rb.rearrange("p (g t) -> p g t", g=4))

            # const views
            t_triub = t_cb[:, CB_TRIU:CB_TRIU + 128]
            t_w2b = t_cb[:, CB_W2:CB_W2 + 1]
            t_onescb = t_cb[:, CB_ONESC:CB_ONESC + 1]
            t_identb = t_cb[:, CB_IDENTB:CB_IDENTB + 128]
            t_fwb = t_cb[:, CB_FWB:CB_FWB + 16]
            t_rcnt = t_cf[:, CF_RCNT:CF_RCNT + 16]
            t_rcnt2 = t_cf[:, CF_RCNT2:CF_RCNT2 + 16]
            t_maskexp = t_cf[:, CF_MASKEXP:CF_MASKEXP + 16]
            t_b1 = t_cf[:, CF_B1:CF_B1 + 1]
            t_sutri16 = t_cf[0:16, CF_SUTRI16:CF_SUTRI16 + 16]
            t_ltri16 = t_cf[0:16, CF_LTRI16:CF_LTRI16 + 16]
            t_onescf = t_cf[:, CF_ONESC:CF_ONESC + 1]
            t_identf = t_cf[:, CF_IDENT:CF_IDENT + 128]
            t_tril = t_cr[:, CR_TRIL:CR_TRIL + 128]
            t_finalw = t_cr[:, CR_FINALW:CR_FINALW + 16]

            def csel_ap(i):
                # strict-lower-than-i selector col, broadcast to (16,128)
                sl = t_cb[0:16, CB_SUTRI16B + i:CB_SUTRI16B + i + 1]
                return bass.AP(tensor=sl.tensor, offset=sl.offset,
                               ap=[[sl.ap[0][0], 16], [0, 128]])

            def bcast16(sb):
                # (16,1) sbuf column -> (16,128) free-broadcast lhsT
                return bass.AP(tensor=sb.tensor, offset=sb.offset,
                               ap=[[sb.ap[0][0], 16], [0, 128]])

            # ------- phase-1 chunk totals (tall-skinny) + carry rows -------
            totC1 = ps_tot.tile([128, 4, 16], f32, tag="tX")
            totC2 = ps_tot.tile([128, 4, 16], f32, tag="tY")
            for i in range(NCH):
                for k in range(4):
                    nc.tensor.matmul(totC1[:, k, i:i + 1],
                                     t_xT[:, i, 128 * k:128 * (k + 1)],
                                     t_onescb, start=True, stop=True)
                    nc.tensor.matmul(totC2[:, k, i:i + 1],
                                     x2[:, i, 128 * k:128 * (k + 1)],
                                     t_onescb, start=True, stop=True)
            totC1b = colp.tile([128, 4, 16], bf16)
            nc.scalar.activation(totC1b[:], totC1[:], AF.Copy)
            totC2b = colp.tile([128, 4, 16], bf16)
            nc.scalar.activation(totC2b[:], totC2[:], AF.Copy)
            trowC_ps = ps_tot.tile([16, 8, 128], bf16, tag="tX")
            for k in range(4):
                nc.tensor.matmul(trowC_ps[:, k, :], totC1b[:, k, :],
                                 t_identb, is_transpose=True,
                                 start=True, stop=True)
            for k in range(4):
                nc.tensor.matmul(trowC_ps[:, 4 + k, :], totC2b[:, k, :],
                                 t_identb, is_transpose=True,
                                 start=True, stop=True)
            trowC = rowp.tile([16, 8, 128], bf16)
            nc.scalar.activation(trowC[:], trowC_ps[:], AF.Copy)
            trowC1 = trowC[:, 0:4, :]
            trowC2 = trowC[:, 4:8, :]

            # ---------------- phase 1 + MLP ----------------
            nats = [None] * NCH
            for i in range(NCH):
                s1 = ps_s.tile([128, C], f32, tag="sA")
                nc.tensor.matmul(s1[:], t_triub, t_xT[:, i, :],
                                 start=True, stop=(i == 0))
                s2 = ps_s1.tile([128, C], f32, tag="sB")
                nc.tensor.matmul(s2[:], t_triub, x2[:, i, :],
                                 start=True, stop=(i == 0))
                if i > 0:
                    nc.tensor.matmul(s1[:], csel_ap(i), trowC1,
                                     start=False, stop=True)
                    nc.tensor.matmul(s2[:], csel_ap(i), trowC2,
                                     start=False, stop=True)
                sqm = sqmp.tile([128, C], bf16, tag="sqm")
                nc.gpsimd.scalar_tensor_tensor(sqm[:], s1[:],
                                               t_rcnt2[:, i:i + 1], s1[:],
                                               op0=OP.mult, op1=OP.mult)
                var1 = v1p.tile([128, C], bf16, tag="v1")
                nc.vector.scalar_tensor_tensor(var1[:], s2[:],
                                               t_rcnt[:, i:i + 1], sqm[:],
                                               op0=OP.mult, op1=OP.subtract)
                if i == 0:
                    # only chunk 0 can see true variance near 0 (count<128)
                    nc.vector.tensor_scalar_max(var1[:], var1[:], EPSC)
                cs = csp.tile([128, C], bf16, tag="cs")
                nc.scalar.activation(cs[:], var1[:], AF.Sqrt)
                nat = natp.tile([128, 4, 128], bf16, tag="nat")
                nc.sync.dma_start_transpose(nat[:], cs[:])
                nats[i] = nat

            # ---------------- MLP (fills the tanh gap) ----------------
            Pm_sb = big.tile([128, T], f32r)
            zeros512 = consts.tile([128, 512], f32)
            nc.vector.memset(zeros512[:], 0.0)
            hpres = [None] * 4
            for g in range(4):
                pm_ps = ps_ca.tile([A, 512], f32, tag="cA")
                for cb in range(4):
                    nc.tensor.matmul(pm_ps[:], t_w1[:, 4 + cb, :],
                                     t_xN[:, cb, 512 * g:512 * (g + 1)],
                                     start=(cb == 0), stop=(cb == 3))
                sl = Pm_sb[:, 512 * g:512 * (g + 1)]
                init = (0.0 if g == 0
                        else Pm_sb[:, 512 * g - 1:512 * g].bitcast(f32))
                nc.vector.tensor_tensor_scan(sl, pm_ps[:], zeros512[:],
                                             initial=init,
                                             op0=OP.add, op1=OP.add)
                nc.vector.tensor_mul(sl, sl.bitcast(f32), rbp[:, g, :])
                ph = ps_tot.tile([A, 512], f32, tag=("tX" if g % 2 == 0
                                                     else "tY"))
                for cb in range(4):
                    nc.tensor.matmul(ph[:], t_w1[:, cb, :],
                                     t_xN[:, cb, 512 * g:512 * (g + 1)],
                                     start=(cb == 0), stop=False)
                for k in range(4):
                    for cb in range(4):
                        nc.tensor.matmul(
                            ph[:, 128 * k:128 * (k + 1)],
                            t_w1[:, 8 + cb, :],
                            nats[4 * g + k][:, cb, :],
                            start=False, stop=(cb == 3))
                hpre = hp.tile([A, 512], f32, tag="hpre")
                nc.gpsimd.scalar_tensor_tensor(
                    hpre[:], ph[:], t_b1,
                    Pm_sb[:, 512 * g:512 * (g + 1)].bitcast(f32),
                    op0=OP.add, op1=OP.add)
                hpres[g] = hpre

            # E accumulates in its own PSUM tile
            eEp = ps_cb.tile([128, NCH], f32, tag="cB")
            eTf = colp.tile([128, NCH], f32)
            eTb = colp.tile([128, NCH], bf16)
            nc.vector.memset(eTb[:], 0.0)
            etot = colp.tile([16, 1], f32)
            R_col = colp.tile([128, NCH], f32)
            # loop A: tanh/scores/exp + E/R per group
            for g in range(4):
                h = hh.tile([A, 512], bf16, tag="h_sb")
                nc.scalar.activation(h[:], hpres[g][:], AF.Tanh)
                ps_sc = ps_ca.tile([128, 4], f32, tag="cA")
                for k in range(4):
                    nc.tensor.matmul(
                        ps_sc[:, k:k + 1],
                        h[:, 128 * k:128 * (k + 1)],
                        t_w2b, start=True, stop=True)
                nc.vector.tensor_add(eTf[:, 4 * g:4 * g + 4],
                                     ps_sc[:],
                                     t_maskexp[:, 4 * g:4 * g + 4])
                nc.scalar.activation(
                    eTb[:, 4 * g:4 * g + 4], eTf[:, 4 * g:4 * g + 4], AF.Exp)
                ps_et = ps_ca.tile([16, 1], f32, tag="cA")
                nc.tensor.matmul(ps_et[:], eTb[:], t_onescb,
                                 start=True, stop=True)
                nc.vector.tensor_copy(etot[:], ps_et[:])
                nc.tensor.matmul(eEp[:, 4 * g:4 * g + 4], t_triub,
                                 eTb[:, 4 * g:4 * g + 4],
                                 start=True, stop=False)
                nc.tensor.matmul(eEp[:, 4 * g:4 * g + 4], bcast16(etot),
                                 t_sutri16[:, 4 * g:4 * g + 4],
                                 start=False, stop=True)
                nc.vector.reciprocal(R_col[:, 4 * g:4 * g + 4],
                                     eEp[:, 4 * g:4 * g + 4])

            # e-weighted chunk totals (tall-skinny) for phase-2 carries
            totE1 = ps_tot.tile([128, 4, 16], f32, tag="tX")
            totE2 = ps_tot.tile([128, 4, 16], f32, tag="tY")
            for ii in range(NCH):
                for k in range(4):
                    nc.tensor.matmul(totE1[:, k, ii:ii + 1],
                                     t_xT[:, ii, 128 * k:128 * (k + 1)],
                                     eTb[:, ii:ii + 1],
                                     start=True, stop=True)
                    nc.tensor.matmul(totE2[:, k, ii:ii + 1],
                                     x2[:, ii, 128 * k:128 * (k + 1)],
                                     eTb[:, ii:ii + 1],
                                     start=True, stop=True)

            # R^2 column for the Pool-side phase-2 square
            R2_col = colp.tile([128, NCH], f32)
            nc.vector.tensor_mul(R2_col[:], R_col[:], R_col[:])
            eR = colp.tile([128, NCH], f32)
            nc.vector.tensor_copy(eR[:], eTb[:])

            # ---------------- G (suffix weights for final mean) -----------
            RLr = colp.tile([128, NCH], f32r)
            nc.vector.tensor_mul(RLr[:], R_col[:], t_finalw.bitcast(f32))
            ps_G = ps_cb.tile([128, NCH], f32, tag="cB")
            nc.tensor.matmul(ps_G[:], t_tril, RLr[:], start=True, stop=False)
            ps_rt = ps_ca.tile([16, 1], f32, tag="cA")
            nc.tensor.matmul(ps_rt[:], RLr[:].bitcast(f32), t_onescf,
                             start=True, stop=True)
            rtot = colp.tile([16, 1], f32)
            nc.vector.tensor_copy(rtot[:], ps_rt[:])
            nc.tensor.matmul(ps_G[:], bcast16(rtot), t_ltri16, start=False,
                             stop=True)
            wG = colp.tile([128, NCH], bf16)
            nc.vector.tensor_mul(wG[:], eTb[:], ps_G[:])

            # phase-2 carry rows from e-weighted totals
            totE1b = colp.tile([128, 4, 16], bf16)
            nc.scalar.activation(totE1b[:], totE1[:], AF.Copy)
            totE2b = colp.tile([128, 4, 16], bf16)
            nc.scalar.activation(totE2b[:], totE2[:], AF.Copy)
            trowE_ps = ps_tot.tile([16, 8, 128], bf16, tag="tX")
            for k in range(4):
                nc.tensor.matmul(trowE_ps[:, k, :], totE1b[:, k, :],
                                 t_identb, is_transpose=True,
                                 start=True, stop=True)
            for k in range(4):
                nc.tensor.matmul(trowE_ps[:, 4 + k, :], totE2b[:, k, :],
                                 t_identb, is_transpose=True,
                                 start=True, stop=True)
            trowE = rowp.tile([16, 8, 128], bf16)
            nc.scalar.activation(trowE[:], trowE_ps[:], AF.Copy)
            trowE1 = trowE[:, 0:4, :]
            trowE2 = trowE[:, 4:8, :]

            # loop B: weights, phase-2 chains; ws tiles all kept alive
            wsall = big.tile([128, NCH, C], bf16)
            for ii in range(NCH):
                wtri = wtp.tile([128, 128], bf16, tag="wtri")
                nc.vector.tensor_scalar_mul(
                    wtri[:], t_triub, eR[:, ii:ii + 1])
                mp = ps_s.tile([128, C], f32, tag="sA")
                nc.tensor.matmul(mp[:], wtri[:], t_xT[:, ii, :],
                                 start=True, stop=(ii == 0))
                if ii > 0:
                    nc.tensor.matmul(mp[:], csel_ap(ii), trowE1,
                                     start=False, stop=True)
                ap = ps_s1.tile([128, C], f32, tag="sB")
                nc.tensor.matmul(ap[:], wtri[:], x2[:, ii, :],
                                 start=True, stop=(ii == 0))
                if ii > 0:
                    nc.tensor.matmul(ap[:], csel_ap(ii), trowE2,
                                     start=False, stop=True)
                zp = zpp.tile([128, C], bf16, tag="zp")
                nc.gpsimd.scalar_tensor_tensor(zp[:], mp[:],
                                               R2_col[:, ii:ii + 1],
                                               mp[:],
                                               op0=OP.mult, op1=OP.mult)
                var2 = v2b.tile([128, C], bf16, tag="v2")
                nc.vector.scalar_tensor_tensor(var2[:], ap[:],
                                               R_col[:, ii:ii + 1], zp[:],
                                               op0=OP.mult,
                                               op1=OP.subtract)
                nc.vector.tensor_scalar_max(var2[:], var2[:], EPSC)
                nc.scalar.activation(wsall[:, ii, :], var2[:], AF.Sqrt)

            # final reductions: one PSUM column at a time (sequential
            # accumulation groups within the shared zero region)
            outacc = ps_ca.tile([128, 8], f32, tag="cA")
            for k in range(4):
                for ii in range(NCH):
                    nc.tensor.matmul(outacc[:, k:k + 1],
                                     t_xT[:, ii, 128 * k:128 * (k + 1)],
                                     wG[:, ii:ii + 1],
                                     start=(ii == 0),
                                     stop=(ii == NCH - 1))
            for k in range(4):
                for ii in range(NCH):
                    nc.tensor.matmul(outacc[:, 4 + k:5 + k],
                                     wsall[:, ii, 128 * k:128 * (k + 1)],
                                     t_fwb[:, ii:ii + 1],
                                     start=(ii == 0),
                                     stop=(ii == NCH - 1))

            # ---------------- output assembly ----------------
            outsb = colp.tile([128, 8], f32)
            nc.scalar.activation(outsb[:], outacc[:], AF.Copy)
            outps = ps_cb.tile([8, 128], f32, tag="cB")
            nc.tensor.matmul(outps[:], outsb[:], t_identf,
                             is_transpose=True, start=True, stop=True)
            outrow = colp.tile([8, 128], f32)
            nc.scalar.activation(outrow[:], outps[:], AF.Copy)
            d_out_r = d_out.rearrange("a (r p) -> (a r) p", r=8)
            nc.sync.dma_start(d_out_r, outrow[:])

    nc.insert_act_table_loads = lambda: None
    nc.compile()
    return nc


def _host_inputs(x, lengths, w1, b1, w2, b2):
    x = np.asarray(x, np.float32)
    lengths = np.asarray(lengths)
    w1 = np.asarray(w1, np.float32)
    b1 = np.asarray(b1, np.float32)
    w2 = np.asarray(w2, np.float32)
    b2 = np.asarray(b2, np.float32)

    sutri16 = np.triu(np.ones((16, 16), np.float32), 1)

    cstf0 = np.zeros((128, NF), np.float32)
    cstf0[:, CF_B1] = b1
    cstf0[:, CF_EPS] = EPSC
    cstf0[0:16, CF_SUTRI16:CF_SUTRI16 + 16] = sutri16
    cstf0[0:16, CF_LTRI16:CF_LTRI16 + 16] = np.tril(
        np.ones((16, 16), np.float32), -1)
    cstf0[:, CF_ONESC] = 1.0
    cstf0[:, CF_IDENT:CF_IDENT + 128] = np.eye(128, dtype=np.float32)

    cstr = np.zeros((128, NR), np.float32)
    cstr[:, CR_TRIL:CR_TRIL + 128] = np.tril(np.ones((128, 128), np.float32))
    cstr[:, CR_ONESC] = 1.0

    cstb0 = np.zeros((128, NB), np.float32)
    cstb0[:, CB_TRIU:CB_TRIU + 128] = np.triu(np.ones((128, 128), np.float32))
    cstb0[0:16, CB_SUTRI16B:CB_SUTRI16B + 16] = sutri16
    cstb0[:, CB_W2] = w2[:, 0]
    cstb0[:, CB_ONESC] = 1.0
    cstb0[:, CB_IDENTB:CB_IDENTB + 128] = np.eye(128, dtype=np.float32)

    tt = np.arange(T)
    w1b = np.ascontiguousarray(
        w1.reshape(12, 128, A).transpose(1, 0, 2).reshape(128, 12 * A)
    ).astype(BF)

    maps = []
    for b in range(B):
        L = int(lengths[b])
        rcnt = (1.0 / np.minimum(tt + 1, max(L, 1))).astype(np.float32)
        maskexp = (float(b2[0]) +
                   np.where(tt < L, 0.0, NEG)).astype(np.float32)
        finalw = np.where(tt < L, 1.0 / max(L, 1), 0.0).astype(np.float32)
        cstf = cstf0.copy()
        cstf[:, CF_RCNT:CF_RCNT + 16] = rcnt.reshape(NCH, 128).T
        cstf[:, CF_RCNT2:CF_RCNT2 + 16] = (rcnt * rcnt).reshape(NCH, 128).T
        cstf[:, CF_MASKEXP:CF_MASKEXP + 16] = maskexp.reshape(NCH, 128).T
        cstr_b = cstr.copy()
        cstr_b[:, CR_FINALW:CR_FINALW + 16] = finalw.reshape(NCH, 128).T
        cstb = cstb0.copy()
        cstb[:, CB_FWB:CB_FWB + 16] = finalw.reshape(NCH, 128).T
        maps.append({
            "xT": np.ascontiguousarray(x[b].T).astype(BF),
            "xN": np.ascontiguousarray(x[b]).astype(BF),
            "w1b": w1b,
            "cstf": cstf,
            "cstr": cstr_b,
            "cstb": cstb.astype(BF),
            "rcntb": np.ascontiguousarray(
                np.broadcast_to(rcnt[None, :], (128, T))).astype(BF),
        })
    return maps


def kernel(x, lengths, w1, b1, w2, b2):
    from concourse.bass_utils import run_bass_kernel_spmd

    if "nc" not in _CACHE:
        _CACHE["nc"] = _build()
    nc = _CACHE["nc"]
    maps = _host_inputs(x, lengths, w1, b1, w2, b2)
    res = run_bass_kernel_spmd(nc, maps, list(range(B))).results
    out = np.stack([res[b]["out"][0] for b in range(B)], axis=0)
    return out.astype(np.float32)
